# revision 72
# baseline (speedup 1.0000x reference)
"""Trainium2 Bass kernel for nn_CapsuleLayer_9852654977072.

The reference module collapses mathematically: the routing loop's coupling
logits `b` stay zero (faithfully-reproduced bug in the original torch code),
so routing coefficients are a fixed spatial map s(h,w) = 1/(8*cnt(h,w)) where
cnt is the 5x5 box-count inside the image. The whole module is therefore:

    praw = conv2d(u as [N,64,H,W], Wd as [128,64,5,5], pad=2)
    v    = praw * sqrt(u2) / (c + u2)        # u2 = sum_z1 praw^2 (groups of 16)
    out[n,t1,z1,h,w] = v                     # c = 1/s^2 spatial map

(The eps inside the reference's sqrt(n2+1e-9) is negligible at these
magnitudes; n2/sqrt(n2+eps) == sqrt(n2) to ~1e-7 relative.)

Device strategy (8 cores, SPMD): shard (batch n in 0..3) x (row-half in 0..1).
Each core computes all 128 output channels for 64 rows of one image.

Conv: inputs shipped as XA/XC fp16 whose partition halves hold u shifted by
(+0row,+1row) and (+2row+0col,+2row+1col), columns padded by 2, stored as 4
overlapping SBUF row-tiles (12/12/20/20/20) so each conv block depends on
exactly one tile and block 0 can start after ~1.3 MB of DMA; XA and XC
halves ship as one tensor, one DMA per tile (descriptor processing is
serial, ~0.65us each).  Per 4-row block, 13 PSUM-accumulated fp16 matmuls
(N=512, full PE rate, all K=128 so FWL hides LDWEIGHTS; tap 13 zero-padded)
cover all 25 taps.

Squash: square (ACT, ->fp16) -> "fat block-diag" matmul bd128[k,m] =
(k//16==m//16) which yields u2 already broadcast over all 128 channel
partitions (no expand matmul) -> sqrt [ACT] / +c, recip-approx, mul [DVE] ->
v = F * praw(PSUM) [DVE] -> fp16 DMA out (host casts back to fp32).  The
spatial map c = A(row)*B(col) is separable: two 1-D vectors DMA-broadcast
over partitions (48 KB) and the map built by stride-0-broadcast DVE muls.

Software pipeline per step b (engine program order, steady state):
    ACT: sq(b-1), sqrt(b-2)
    PE : conv(b) x13, bd(b-1)          <- dense 213ns/MM stream
    DVE: add(b-2), recip(b-2), F(b-2), v(b-2)
Startup: dummy PE matmuls warm the HAM clock gate during the DMA fill; late
input tiles load behind engine-ordered dummy reads so they don't
round-robin-steal DMA bandwidth from tile 0; bd(14) is hoisted into the
middle of conv(15) so the last chains overlap the final conv block.
"""

import numpy as np

T0, Z0, T1, Z1, KK, PAD = 4, 16, 8, 16, 5, 2
N, H, W_SP = 4, 128, 128
CIN, COUT = T0 * Z0, T1 * Z1  # 64, 128
N_CORES = 8
ROWS = 64          # output rows per core
XROWS = 68         # input rows incl. halo
XCOLS = 132        # 128 + 2*PAD
BLK = 4            # output rows per block
N_BLKS = ROWS // BLK
# input SBUF tiles: (row_start, n_rows, first_block, n_blocks); overlapping so
# each conv block reads exactly one tile, and tile 0 is small (startup path)
_TILES = [(0, 12, 0, 2), (8, 12, 2, 2), (16, 20, 4, 4),
          (32, 20, 8, 4), (48, 20, 12, 4)]
_BLK2TILE = {}
for _ti, (_rs, _nr, _b0, _nb) in enumerate(_TILES):
    for _b in range(_b0, _b0 + _nb):
        _BLK2TILE[_b] = (_ti, _rs)

# conv matmul j -> (source, row_off, col_off); weights match in _weight_tiles
_MM_SLICES = (
    [('XA', dy + 2, dx + 2) for dy in (-2, 0) for dx in (-2, -1, 0, 1, 2)]
    + [('XC', 2, 0), ('XC', 2, 2), ('XC', 2, 4)]
)

_CACHE = {}


def _weight_tiles(W):
    Wd = W.transpose(1, 0, 2, 3, 4).reshape(COUT, CIN, KK, KK)
    wl = np.zeros((128, 13, 128), np.float32)  # [k, j, m]
    j = 0
    for dy in (-2, 0):
        for dx in (-2, -1, 0, 1, 2):
            wl[0:64, j, :] = Wd[:, :, dy + 2, dx + 2].T
            wl[64:128, j, :] = Wd[:, :, dy + 3, dx + 2].T
            j += 1
    for dx0 in (-2, 0):
        wl[0:64, j, :] = Wd[:, :, 4, dx0 + 2].T
        wl[64:128, j, :] = Wd[:, :, 4, dx0 + 3].T
        j += 1
    wl[0:64, j, :] = Wd[:, :, 4, 4].T  # single tap (2,2); hi partitions stay 0
    return wl.astype(np.float16)


def _inputs_core(x, half):
    """x: [64, H, W] one image channel-major. Returns XA, XC fp16 [128,68,132]."""
    base = half * 64 - 2
    XA = np.zeros((128, XROWS, XCOLS), np.float16)
    XC = np.zeros((128, XROWS, XCOLS), np.float16)

    def fill(dst, roff, c0, c1):
        lo, hi = max(0, -(base + roff)), min(XROWS, H - base - roff)
        dst[:, lo:hi, c0:c1] = x[:, base + roff + lo:base + roff + hi, :]

    fill(XA[0:64], 0, 2, 130)
    fill(XA[64:128], 1, 2, 130)
    fill(XC[0:64], 2, 2, 130)
    fill(XC[64:128], 2, 1, 129)
    return XA, XC


def _c_vecs(half):
    """c(h,w) = 1/s^2 = (8*cnt_r*cnt_c)^2 = A(h)*B(w), A = 64*cnt_r^2."""
    idx = np.arange(H)
    cnt = (np.minimum(idx + 2, H - 1) - np.maximum(idx - 2, 0) + 1).astype(np.float64)
    A = (64.0 * cnt[half * 64:(half + 1) * 64] ** 2).astype(np.float16)
    B = (cnt ** 2).astype(np.float16)
    return A.reshape(1, ROWS), B.reshape(1, W_SP)


def _block_diag128():
    k = np.arange(128)
    return ((k[:, None] // 16) == (k[None, :] // 16)).astype(np.float16)


def build_nc():
    import concourse.bass as bass
    import concourse.bacc as bacc
    import concourse.mybir as mybir
    import concourse.tile as tile

    f32 = mybir.dt.float32
    f16 = mybir.dt.float16
    AF = mybir.ActivationFunctionType

    nc = bacc.Bacc(None, target_bir_lowering=False)
    xz_d = nc.dram_tensor("xz", [128, 2 * XROWS * XCOLS], f16,
                          kind="ExternalInput")
    wl_d = nc.dram_tensor("wl", [128, 13 * 128], f16, kind="ExternalInput")
    bd_d = nc.dram_tensor("bd", [128, 128], f16, kind="ExternalInput")
    ca_d = nc.dram_tensor("ca", [1, ROWS], f16, kind="ExternalInput")
    cb_d = nc.dram_tensor("cb", [1, W_SP], f16, kind="ExternalInput")
    out_d = nc.dram_tensor("out", [128, ROWS * 128], f16, kind="ExternalOutput")

    with tile.TileContext(nc) as tc:
        with (
            tc.tile_pool(name="consts", bufs=1) as consts,
            tc.tile_pool(name="work", bufs=4) as work,
            tc.tile_pool(name="small", bufs=4) as small,
            tc.tile_pool(name="pp", bufs=4, space="PSUM") as pp,
            tc.tile_pool(name="py", bufs=4, space="PSUM") as py,
        ):
            xz_src = xz_d.ap().rearrange("p (s r c) -> p s r c",
                                         s=2, c=XCOLS)

            # startup-critical loads first: wl + input tile 0 (one DMA per
            # tile covers both the XA and XC halves: descriptor processing
            # is ~0.65us each and serial on the queue, so fewer is faster)
            wl = consts.tile([128, 13, 128], f16)
            nc.sync.dma_start(
                out=wl, in_=wl_d.ap().rearrange("p (j m) -> p j m", m=128))
            xz_t = []
            for k, (rs, nr, _, _2) in enumerate(_TILES):
                xz_t.append(consts.tile([128, 2, nr, XCOLS], f16,
                                        name=f"xz{k}", tag=f"xz{k}"))
            rs0, nr0 = _TILES[0][0], _TILES[0][1]
            nc.sync.dma_start(out=xz_t[0],
                              in_=xz_src[:, :, rs0:rs0 + nr0, :])
            bd = consts.tile([128, 128], f16)
            nc.sync.dma_start(out=bd, in_=bd_d.ap())
            dum = consts.tile([1, 8], f16)

            # c map is separable: c = A(row)*B(col).  Ship the two tiny
            # vectors broadcast over partitions (48 KB total), then build the
            # full [128, 64, 128] map with stride-0-broadcast DVE multiplies
            # while DVE is idle during the startup DMA fill.
            ca_sb = consts.tile([128, ROWS], f16)
            ca_ap = ca_d.ap()
            nc.gpsimd.dma_start(
                out=ca_sb, in_=bass.AP(tensor=ca_ap.tensor, offset=ca_ap.offset,
                                       ap=[[0, 128], [1, ROWS]]))
            cb_sb = consts.tile([128, W_SP], f16)
            cb_ap = cb_d.ap()
            nc.gpsimd.dma_start(
                out=cb_sb, in_=bass.AP(tensor=cb_ap.tensor, offset=cb_ap.offset,
                                       ap=[[0, 128], [1, W_SP]]))
            cm_sb = consts.tile([128, ROWS, 128], f16)

            def build_cm(r0, r1):
                nc.vector.tensor_mul(
                    cm_sb[:, r0:r1, :],
                    bass.AP(tensor=ca_sb.tensor, offset=ca_sb.offset + r0,
                            ap=list(ca_sb.ap[:1]) + [[1, r1 - r0], [0, 128]]),
                    bass.AP(tensor=cb_sb.tensor, offset=cb_sb.offset,
                            ap=list(cb_sb.ap[:1]) + [[0, r1 - r0], [1, 128]]))

            def load_tile(k, queue=None, anchor=True):
                # ACT dummy reads give the sync-queue DMAs an engine-ordered
                # WAR dependency so they fire mid-run instead of stealing
                # round-robin DMA bandwidth from tile 0 / wl during startup.
                if anchor:
                    nc.scalar.activation(dum[0:1, 0:1],
                                         xz_t[k][0:1, 0, 0, 0:1],
                                         AF.Copy, bias=0.0)
                rs, nr = _TILES[k][0], _TILES[k][1]
                eng = queue if queue is not None else nc.sync
                eng.dma_start(out=xz_t[k],
                              in_=xz_src[:, :, rs:rs + nr, :])

            out_v = out_d.ap().rearrange("p (r c) -> p r c", c=128)

            # HAM warm-up: PE-only dummy matmuls during the input-DMA fill so
            # the clock gate opens right as the first real conv block starts.
            wtmp = consts.tile([128, 256], f16)
            nc.vector.memset(wtmp[:], 0.125)
            pwarm = pp.tile([128, BLK, 128], f32, tag="p_ps")

            def pe_release(k):
                # a PE matmul that READS xz_t[k] (garbage, result unused):
                # the tile's reload DMA gets a WAR dependency on it, so the
                # transfer starts only when the warm-up stream reaches this
                # point (~when tile 0's own transfer completes).
                nc.tensor.matmul(pwarm[:, 2:3, :], wtmp[:, 0:128],
                                 xz_t[k][:, 0, 0:1, 0:128],
                                 start=True, stop=True)

            for _ in range(16):
                nc.tensor.matmul(pwarm[:, 0:2, :], wtmp[:, 0:128], wtmp[:],
                                 start=True, stop=True)
            pe_release(1)
            for _ in range(4):
                nc.tensor.matmul(pwarm[:, 0:2, :], wtmp[:, 0:128], wtmp[:],
                                 start=True, stop=True)
            pe_release(2)

            # DVE during the fill: build the c map (runs at DVE 4x).
            build_cm(0, 32)
            build_cm(32, ROWS)
            load_tile(1, anchor=False)
            load_tile(2, anchor=False)

            st = {}

            def conv(blk, mid_pe=None):
                ti, rs = _BLK2TILE[blk]
                lr0 = blk * BLK - rs
                xzk = xz_t[ti]
                p_ps = pp.tile([128, BLK, 128], f32, tag="p_ps")
                for j, (src, roff, coff) in enumerate(_MM_SLICES):
                    if j == 4 and mid_pe is not None:
                        mid_pe()
                    si = 0 if src == 'XA' else 1
                    nc.tensor.matmul(
                        p_ps[:], wl[:, j, :],
                        xzk[:, si, lr0 + roff:lr0 + roff + BLK,
                            coff:coff + 128],
                        start=(j == 0), stop=(j == 12))
                st[blk] = {"p": p_ps}

            def act_sq(blk):
                s = st[blk]
                psq = work.tile([128, BLK, 128], f16, tag="psq")
                nc.scalar.activation(psq[:], s["p"][:], AF.Square)
                s["psq"] = psq

            def pe_bd(blk):
                s = st[blk]
                y_ps = py.tile([128, BLK, 128], f32, tag="y_ps")
                nc.tensor.matmul(y_ps[:], bd[:], s["psq"][:],
                                 start=True, stop=True)
                s["y"] = y_ps

            def act_sqrt(blk):
                s = st[blk]
                a_t = small.tile([128, BLK, 128], f16, tag="a")
                nc.scalar.activation(a_t[:], s["y"][:], AF.Sqrt)
                s["a"] = a_t

            def dve_rest(blk):
                s = st.pop(blk)
                r0 = blk * BLK
                # F = sqrt(u2) / (c + u2); v = praw * F
                d_t = small.tile([128, BLK, 128], f32, tag="d")
                nc.vector.tensor_add(d_t[:], s["y"][:], cm_sb[:, r0:r0 + BLK, :])
                r_t = small.tile([128, BLK, 128], f32, tag="r")
                nc.vector.reciprocal_approx_fast(r_t[:], d_t[:])
                F_t = small.tile([128, BLK, 128], f16, tag="F")
                nc.vector.tensor_mul(F_t[:], s["a"][:], r_t[:])
                v_t = work.tile([128, BLK, 128], f16, tag="v")
                nc.vector.tensor_mul(v_t[:], F_t[:], s["p"][:])
                nc.sync.dma_start(out=out_v[:, r0:r0 + BLK, :], in_=v_t[:])

            _LOAD_AT = {5: 3, 9: 4}
            L = N_BLKS - 1
            for b in range(N_BLKS):
                if 1 <= b:
                    act_sq(b - 1)
                if b in _LOAD_AT:
                    load_tile(_LOAD_AT[b])
                if 2 <= b:
                    act_sqrt(b - 2)
                if b == L:
                    # pull bd(L-1) into the middle of the last conv so
                    # chain(L-1) overlaps it instead of trailing it
                    conv(b, mid_pe=lambda: pe_bd(L - 1))
                    act_sqrt(L - 1)
                else:
                    conv(b)
                    if 1 <= b:
                        pe_bd(b - 1)
                if 2 <= b:
                    dve_rest(b - 2)
                if b == L:
                    dve_rest(L - 1)
            # epilogue: drain the last block
            act_sq(L)
            pe_bd(L)
            act_sqrt(L)
            dve_rest(L)

    nc.compile()
    return nc


def _prep_in_maps(u, W):
    x = u.reshape(N, CIN, H, W_SP)
    wl = _weight_tiles(W).reshape(128, 13 * 128)
    bd = _block_diag128()
    in_maps = []
    for core in range(N_CORES):
        n, half = core // 2, core % 2
        XA, XC = _inputs_core(x[n], half)
        ca, cb = _c_vecs(half)
        in_maps.append({
            "xz": np.ascontiguousarray(np.concatenate(
                [XA.reshape(128, -1), XC.reshape(128, -1)], axis=1)),
            "wl": wl,
            "bd": bd,
            "ca": ca,
            "cb": cb,
        })
    return in_maps


def run(u, W, trace=False):
    """Returns (out [N,T1,Z1,H,W] f32, BassKernelResults)."""
    from concourse.bass_utils import run_bass_kernel_spmd

    if "nc" not in _CACHE:
        _CACHE["nc"] = build_nc()
    nc = _CACHE["nc"]
    in_maps = _prep_in_maps(np.asarray(u, np.float32), np.asarray(W, np.float32))
    res = run_bass_kernel_spmd(nc, in_maps, list(range(N_CORES)), trace=trace)
    out = np.empty((N, T1, Z1, H, W_SP), np.float32)
    for core in range(N_CORES):
        n, half = core // 2, core % 2
        o = res.results[core]["out"].astype(np.float32).reshape(T1, Z1, ROWS, 128)
        out[n, :, :, half * 64:(half + 1) * 64, :] = o
    return out, res


def kernel(u, W):
    out, _ = run(u, W, trace=False)
    return out
